# revision 12
# baseline (speedup 1.0000x reference)
"""Cayley-circulant SSM layer as a Trainium2 Bass kernel.

Math: h_t = W h_{t-1} + B u_t, y_t = C h_t + D u_t, where W is a real
orthogonal circulant (Cayley transform of a skew-circulant) diagonalized
by the DFT with unit-modulus eigenvalues lambda_k = e^{i theta_k}.

Device algorithm (frequency-domain associative scan):
  1. Fold the rfft into B and C on the host (weight preprocessing):
     buhat_t = (F B) u_t restricted to 512 packed real frequency
     channels (Hermitian symmetry; DC and Nyquist share channel 0 as
     (re, im) with theta=0).
  2. The recurrence hhat_t = lambda * hhat_{t-1} + buhat_t becomes,
     with z_t = conj(lambda)^t * buhat_t, a plain cumulative sum:
     hhat_t = lambda^t * cumsum(z)_t.  |lambda|=1 so this is exact.
  3. y_t = Re(G hhat_t) = Ar @ hhat_r + Ai @ hhat_i + D u_t.

Per-core layout (data-parallel over batch, 1 row per NeuronCore):
  MM1  (PE, fp32r):  bu_{r,i}(k,t) = BrT/BiT.T @ uT          (d contracted)
  twist (DVE):       m1 = c*bur, m2 = s*bui, m3 = c*bui, m4 = s*bur
  scan  (DVE):       Sr = cumsum(m1+m2), Si = cumsum(m3-m4)   (fused)
  untwist (DVE):     hr = c*Sr - s*Si, hi = c*Si + s*Sr       (fp32r out)
  MM3  (PE, fp32r):  yT(d,t) = ArT.T @ hr + AiT.T @ hi        (k contracted)
cos/sin tables are host-precomputed in float64 per (k, t).
"""

import numpy as np

import concourse.bass as bass  # noqa: F401  (registers engine types)
import concourse.mybir as mybir
import concourse.tile as tile
from concourse import bacc
from concourse.bass_utils import run_bass_kernel_spmd

BATCH, SEQ, DM, NSTATE = 8, 2048, 1024, 1024
K = NSTATE // 2          # packed real frequency channels
P = 128                  # partitions
TT = 512                 # t-tile width (one PSUM bank of fp32)
ND, NK, NT = DM // P, K // P, SEQ // TT

_f32 = mybir.dt.float32
_f32r = mybir.dt.float32r
_add = mybir.AluOpType.add
_sub = mybir.AluOpType.subtract
_mul = mybir.AluOpType.mult

_COMPILED = None


def _emit(tc, nc, dr):
    ut_d, brt_d, bit_d, art_d, ait_d, cs_d, sn_d, yt_d = dr
    NP = NK // 2      # k-tile pairs: SBUF elementwise ops run on [P, 2*TT]
    W = 2 * TT
    with (
        tc.tile_pool(name="ust", bufs=2) as ust,
        tc.tile_pool(name="wb", bufs=1) as wb,
        tc.tile_pool(name="wa", bufs=1) as wa,
        tc.tile_pool(name="tbl", bufs=2) as tbl,
        tc.tile_pool(name="hbuf", bufs=2) as hbuf,
        tc.tile_pool(name="scr", bufs=2) as scr,
        tc.tile_pool(name="sbuf_s", bufs=2) as sbuf_s,
        tc.tile_pool(name="ini", bufs=2) as ini,
        tc.tile_pool(name="psA", bufs=2, space="PSUM") as psA,
        tc.tile_pool(name="psY", bufs=4, space="PSUM") as psY,
    ):
        art = [None] * NK
        ait = [None] * NK
        brw = [[None] * ND for _ in range(NK)]
        biw = [[None] * ND for _ in range(NK)]
        # per-k-tile scan carry (last cumsum column of the previous t-tile)
        ir = [ini.tile([P, 1], _f32, name=f"ir{kt}", tag=f"ir{kt}")
              for kt in range(NK)]
        ii = [ini.tile([P, 1], _f32, name=f"ii{kt}", tag=f"ii{kt}")
              for kt in range(NK)]

        for tt in range(NT):
            ts = slice(tt * TT, (tt + 1) * TT)
            us = []
            for di in range(ND):
                t = ust.tile([P, TT], _f32r, tag=f"us{di}")
                nc.sync.dma_start(t[:], ut_d[di, :, ts])
                us.append(t)
            hr2, hi2 = [], []
            for p in range(NP):
                kt0, kt1 = 2 * p, 2 * p + 1
                if tt == 0:
                    for kt in (kt0, kt1):
                        for di in range(ND):
                            tb = wb.tile([P, P], _f32r, name=f"brw{kt}_{di}",
                                         tag=f"brw{kt}_{di}")
                            nc.sync.dma_start(
                                tb[:], brt_d[di, :, kt * P:(kt + 1) * P])
                            brw[kt][di] = tb
                            ti = wb.tile([P, P], _f32r, name=f"biw{kt}_{di}",
                                         tag=f"biw{kt}_{di}")
                            nc.sync.dma_start(
                                ti[:], bit_d[di, :, kt * P:(kt + 1) * P])
                            biw[kt][di] = ti
                c2 = tbl.tile([P, W], _f32, tag="c2")
                s2 = tbl.tile([P, W], _f32, tag="s2")
                nc.sync.dma_start(c2[:, :TT], cs_d[kt0, :, ts])
                nc.sync.dma_start(c2[:, TT:], cs_d[kt1, :, ts])
                nc.sync.dma_start(s2[:, :TT], sn_d[kt0, :, ts])
                nc.sync.dma_start(s2[:, TT:], sn_d[kt1, :, ts])

                sr2 = sbuf_s.tile([P, W], _f32, tag="sr2")
                si2 = sbuf_s.tile([P, W], _f32, tag="si2")
                for h, kt in ((0, kt0), (1, kt1)):
                    sl = slice(h * TT, (h + 1) * TT)
                    pbr = psA.tile([P, TT], _f32, tag="pbr")
                    pbi = psA.tile([P, TT], _f32, tag="pbi")
                    for di in range(ND):
                        nc.tensor.matmul(pbr[:], brw[kt][di][:], us[di][:],
                                         start=(di == 0), stop=(di == ND - 1))
                        nc.tensor.matmul(pbi[:], biw[kt][di][:], us[di][:],
                                         start=(di == 0), stop=(di == ND - 1))
                    m1 = scr.tile([P, TT], _f32, tag="m1")
                    m2 = scr.tile([P, TT], _f32, tag="m2")
                    nc.vector.tensor_tensor(m1[:], c2[:, sl], pbr[:], _mul)
                    nc.vector.tensor_tensor(m2[:], s2[:, sl], pbi[:], _mul)
                    init_r = 0.0 if tt == 0 else ir[kt][:]
                    nc.vector.tensor_tensor_scan(sr2[:, sl], m1[:], m2[:],
                                                 init_r, _add, _add)
                    m3 = scr.tile([P, TT], _f32, tag="m1")
                    m4 = scr.tile([P, TT], _f32, tag="m2")
                    nc.vector.tensor_tensor(m3[:], c2[:, sl], pbi[:], _mul)
                    nc.vector.tensor_tensor(m4[:], s2[:, sl], pbr[:], _mul)
                    init_i = 0.0 if tt == 0 else ii[kt][:]
                    nc.vector.tensor_tensor_scan(si2[:, sl], m3[:], m4[:],
                                                 init_i, _add, _sub)
                    if tt < NT - 1:
                        e = (h + 1) * TT
                        nc.scalar.copy(ir[kt][:], sr2[:, e - 1:e])
                        nc.scalar.copy(ii[kt][:], si2[:, e - 1:e])

                # untwist on [P, 2*TT]: real part on DVE, imag on GPSIMD
                w1 = scr.tile([P, W], _f32, tag="w1")
                w2 = scr.tile([P, W], _f32, tag="w2")
                hrp = hbuf.tile([P, W], _f32r, name=f"hr2_{p}", tag=f"hr2_{p}")
                hip = hbuf.tile([P, W], _f32r, name=f"hi2_{p}", tag=f"hi2_{p}")
                nc.vector.tensor_tensor(w1[:], c2[:], sr2[:], _mul)
                nc.vector.tensor_tensor(w2[:], s2[:], si2[:], _mul)
                nc.vector.tensor_tensor(hrp[:], w1[:], w2[:], _sub)
                w3 = scr.tile([P, W], _f32, tag="w3", bufs=1)
                w4 = scr.tile([P, W], _f32, tag="w4", bufs=1)
                nc.gpsimd.tensor_tensor(w3[:], c2[:], si2[:], _mul)
                nc.gpsimd.tensor_tensor(w4[:], s2[:], sr2[:], _mul)
                nc.gpsimd.tensor_tensor(hip[:], w3[:], w4[:], _add)
                hr2.append(hrp)
                hi2.append(hip)

                if tt == 0:
                    # A-projection weights: needed first at this tt's MM3;
                    # emitted here so they trail the startup-critical loads
                    for kt in (kt0, kt1):
                        ta = wa.tile([P, DM], _f32r, name=f"art{kt}",
                                     tag=f"art{kt}")
                        nc.sync.dma_start(ta[:], art_d[kt])
                        art[kt] = ta
                        ti = wa.tile([P, DM], _f32r, name=f"ait{kt}",
                                     tag=f"ait{kt}")
                        nc.sync.dma_start(ti[:], ait_d[kt])
                        ait[kt] = ti

            for di in range(ND):
                py = psY.tile([P, TT], _f32, tag="py")
                for p in range(NP):
                    for h, kt in ((0, 2 * p), (1, 2 * p + 1)):
                        sl = slice(h * TT, (h + 1) * TT)
                        nc.tensor.matmul(py[:], art[kt][:, di * P:(di + 1) * P],
                                         hr2[p][:, sl],
                                         start=(p == 0 and h == 0), stop=False)
                        nc.tensor.matmul(py[:], ait[kt][:, di * P:(di + 1) * P],
                                         hi2[p][:, sl], start=False,
                                         stop=(p == NP - 1 and h == 1))
                ysb = sbuf_s.tile([P, TT], _f32, tag="ysb", bufs=4)
                nc.scalar.copy(ysb[:], py[:])
                nc.sync.dma_start(yt_d[di, :, ts], ysb[:])


def _build():
    nc = bacc.Bacc("TRN2", target_bir_lowering=False, debug=False,
                   num_devices=BATCH)
    ut_d = nc.dram_tensor("ut", [ND, P, SEQ], _f32r, kind="ExternalInput")
    brt_d = nc.dram_tensor("brt", [ND, P, K], _f32r, kind="ExternalInput")
    bit_d = nc.dram_tensor("bit", [ND, P, K], _f32r, kind="ExternalInput")
    art_d = nc.dram_tensor("art", [NK, P, DM], _f32r, kind="ExternalInput")
    ait_d = nc.dram_tensor("ait", [NK, P, DM], _f32r, kind="ExternalInput")
    cs_d = nc.dram_tensor("cs", [NK, P, SEQ], _f32, kind="ExternalInput")
    sn_d = nc.dram_tensor("sn", [NK, P, SEQ], _f32, kind="ExternalInput")
    yt_d = nc.dram_tensor("yt", [ND, P, SEQ], _f32, kind="ExternalOutput")
    with tile.TileContext(nc) as tc:
        _emit(tc, nc, (ut_d, brt_d, bit_d, art_d, ait_d, cs_d, sn_d, yt_d))
    nc.compile()
    return nc


def _host_prep(a_params, B_w, C_w):
    """Fold the DFT into B/C and build phase tables (float64 on host)."""
    n, half = NSTATE, K
    a = a_params.astype(np.float64)
    a_full = np.zeros(n)
    a_full[1:half] = a[:half - 1]
    a_full[half + 1:] = -a[:half - 1][::-1]
    omega = np.fft.fft(a_full).imag
    theta = -2.0 * np.arctan(omega)          # lambda_k = e^{i theta_k}

    Bf = np.fft.fft(B_w.astype(np.float64), axis=0)[:half + 1]      # (513, d)
    G = np.conj(np.fft.fft(C_w.astype(np.float64), axis=1))[:, :half + 1]

    Br = np.empty((K, DM))
    Bi = np.empty((K, DM))
    Br[0], Bi[0] = Bf[0].real, Bf[half].real   # DC + Nyquist packed, theta=0
    Br[1:], Bi[1:] = Bf[1:half].real, Bf[1:half].imag
    Ar = np.empty((DM, K))
    Ai = np.empty((DM, K))
    Ar[:, 0] = (1.0 / n) * G[:, 0].real
    Ai[:, 0] = (1.0 / n) * G[:, half].real
    Ar[:, 1:] = (2.0 / n) * G[:, 1:half].real
    Ai[:, 1:] = -(2.0 / n) * G[:, 1:half].imag
    th = theta[:half].copy()
    th[0] = 0.0

    ang = np.outer(th, np.arange(SEQ, dtype=np.float64))   # (K, SEQ)
    f32 = np.float32

    def tiles(m, p):          # (R, C) -> (R//p, p, C) contiguous f32
        return np.ascontiguousarray(m.reshape(m.shape[0] // p, p, m.shape[1]),
                                    dtype=f32)

    return {
        "brt": tiles(Br.T.copy(), P),       # (8, 128, 512)  BrT[d, k]
        "bit": tiles(Bi.T.copy(), P),
        "art": tiles(Ar.T.copy(), P),       # (4, 128, 1024) ArT[k, d]
        "ait": tiles(Ai.T.copy(), P),
        "cs": tiles(np.cos(ang), P),        # (4, 128, 2048)
        "sn": tiles(np.sin(ang), P),
    }


def _run(u, a_params, B_w, C_w, D, trace=False):
    global _COMPILED
    if _COMPILED is None:
        _COMPILED = _build()
    nc = _COMPILED
    shared = _host_prep(np.asarray(a_params), np.asarray(B_w), np.asarray(C_w))
    u = np.asarray(u)
    in_maps = []
    for b in range(BATCH):
        m = dict(shared)
        m["ut"] = np.ascontiguousarray(
            u[b].T.reshape(ND, P, SEQ), dtype=np.float32)
        in_maps.append(m)
    res = run_bass_kernel_spmd(nc, in_maps, core_ids=list(range(BATCH)),
                               trace=trace)
    y = np.empty((BATCH, SEQ, DM), dtype=np.float32)
    for b in range(BATCH):
        y[b] = res.results[b]["yt"].reshape(DM, SEQ).T
    y += np.asarray(D)[None, None, :] * u
    return y, res


def kernel(u, a_params, B_w, C_w, D):
    y, _ = _run(u, a_params, B_w, C_w, D)
    return y


# revision 13
# speedup vs baseline: 1.1280x; 1.1280x over previous
"""Cayley-circulant SSM layer as a Trainium2 Bass kernel.

Math: h_t = W h_{t-1} + B u_t, y_t = C h_t + D u_t, where W is a real
orthogonal circulant (Cayley transform of a skew-circulant) diagonalized
by the DFT with unit-modulus eigenvalues lambda_k = e^{i theta_k}.

Device algorithm (frequency-domain associative scan):
  1. Fold the rfft into B and C on the host (weight preprocessing):
     buhat_t = (F B) u_t restricted to 512 packed real frequency
     channels (Hermitian symmetry; DC and Nyquist share channel 0 as
     (re, im) with theta=0).
  2. The recurrence hhat_t = lambda * hhat_{t-1} + buhat_t becomes,
     with z_t = conj(lambda)^t * buhat_t, a plain cumulative sum:
     hhat_t = lambda^t * cumsum(z)_t.  |lambda|=1 so this is exact.
  3. y_t = Re(G hhat_t) = Ar @ hhat_r + Ai @ hhat_i + D u_t.

Per-core layout (data-parallel over batch, 1 row per NeuronCore):
  MM1  (PE, fp32r):  bu_{r,i}(k,t) = BrT/BiT.T @ uT          (d contracted)
  twist (DVE):       m1 = c*bur, m2 = s*bui, m3 = c*bui, m4 = s*bur
  scan  (DVE):       Sr = cumsum(m1+m2), Si = cumsum(m3-m4)   (fused)
  untwist (DVE):     hr = c*Sr - s*Si, hi = c*Si + s*Sr       (fp32r out)
  MM3  (PE, fp32r):  yT(d,t) = ArT.T @ hr + AiT.T @ hi        (k contracted)
cos/sin tables are host-precomputed in float64 per (k, t).
"""

import numpy as np

import concourse.bass as bass  # noqa: F401  (registers engine types)
import concourse.mybir as mybir
import concourse.tile as tile
from concourse import bacc
from concourse.bass_utils import run_bass_kernel_spmd

BATCH, SEQ, DM, NSTATE = 8, 2048, 1024, 1024
K = NSTATE // 2          # packed real frequency channels
P = 128                  # partitions
TT = 512                 # t-tile width (one PSUM bank of fp32)
ND, NK, NT = DM // P, K // P, SEQ // TT

_f32 = mybir.dt.float32
_f32r = mybir.dt.float32r
_add = mybir.AluOpType.add
_sub = mybir.AluOpType.subtract
_mul = mybir.AluOpType.mult

_COMPILED = None


def _emit_mm3(nc, psY, sbuf_s, art, ait, hr2, hi2, yt_d, tt):
    ts = slice(tt * TT, (tt + 1) * TT)
    NP = NK // 2
    for di in range(ND):
        py = psY.tile([P, TT], _f32, tag="py")
        for p in range(NP):
            for h, kt in ((0, 2 * p), (1, 2 * p + 1)):
                sl = slice(h * TT, (h + 1) * TT)
                nc.tensor.matmul(py[:], art[kt][:, di * P:(di + 1) * P],
                                 hr2[p][:, sl],
                                 start=(p == 0 and h == 0), stop=False)
                nc.tensor.matmul(py[:], ait[kt][:, di * P:(di + 1) * P],
                                 hi2[p][:, sl], start=False,
                                 stop=(p == NP - 1 and h == 1))
        ysb = sbuf_s.tile([P, TT], _f32, tag="ysb", bufs=4)
        nc.scalar.copy(ysb[:], py[:])
        nc.sync.dma_start(yt_d[di, :, ts], ysb[:])


def _emit(tc, nc, dr):
    ut_d, brt_d, bit_d, art_d, ait_d, cs_d, sn_d, yt_d = dr
    NP = NK // 2      # k-tile pairs: SBUF elementwise ops run on [P, 2*TT]
    W = 2 * TT
    with (
        tc.tile_pool(name="ust", bufs=2) as ust,
        tc.tile_pool(name="wb", bufs=1) as wb,
        tc.tile_pool(name="wa", bufs=1) as wa,
        tc.tile_pool(name="tbl", bufs=2) as tbl,
        tc.tile_pool(name="hbuf", bufs=2) as hbuf,
        tc.tile_pool(name="scr", bufs=2) as scr,
        tc.tile_pool(name="sbuf_s", bufs=2) as sbuf_s,
        tc.tile_pool(name="ini", bufs=2) as ini,
        tc.tile_pool(name="psA", bufs=2, space="PSUM") as psA,
        tc.tile_pool(name="psY", bufs=4, space="PSUM") as psY,
    ):
        art = [None] * NK
        ait = [None] * NK
        brw = [[None] * ND for _ in range(NK)]
        biw = [[None] * ND for _ in range(NK)]
        # per-k-tile scan carry (last cumsum column of the previous t-tile)
        ir = [ini.tile([P, 1], _f32, name=f"ir{kt}", tag=f"ir{kt}")
              for kt in range(NK)]
        ii = [ini.tile([P, 1], _f32, name=f"ii{kt}", tag=f"ii{kt}")
              for kt in range(NK)]

        for tt in range(NT):
            ts = slice(tt * TT, (tt + 1) * TT)
            us = []
            for di in range(ND):
                t = ust.tile([P, TT], _f32r, tag=f"us{di}")
                nc.sync.dma_start(t[:], ut_d[di, :, ts])
                us.append(t)
            hr2, hi2 = [], []
            for p in range(NP):
                kt0, kt1 = 2 * p, 2 * p + 1
                if tt == 0:
                    for kt in (kt0, kt1):
                        for di in range(ND):
                            tb = wb.tile([P, P], _f32r, name=f"brw{kt}_{di}",
                                         tag=f"brw{kt}_{di}")
                            nc.sync.dma_start(
                                tb[:], brt_d[di, :, kt * P:(kt + 1) * P])
                            brw[kt][di] = tb
                            ti = wb.tile([P, P], _f32r, name=f"biw{kt}_{di}",
                                         tag=f"biw{kt}_{di}")
                            nc.sync.dma_start(
                                ti[:], bit_d[di, :, kt * P:(kt + 1) * P])
                            biw[kt][di] = ti
                c2 = tbl.tile([P, W], _f32, tag="c2")
                s2 = tbl.tile([P, W], _f32, tag="s2")
                nc.sync.dma_start(c2[:, :TT], cs_d[kt0, :, ts])
                nc.sync.dma_start(c2[:, TT:], cs_d[kt1, :, ts])
                nc.sync.dma_start(s2[:, :TT], sn_d[kt0, :, ts])
                nc.sync.dma_start(s2[:, TT:], sn_d[kt1, :, ts])

                sr2 = sbuf_s.tile([P, W], _f32, tag="sr2")
                si2 = sbuf_s.tile([P, W], _f32, tag="si2")
                for h, kt in ((0, kt0), (1, kt1)):
                    sl = slice(h * TT, (h + 1) * TT)
                    pbr = psA.tile([P, TT], _f32, tag="pbr")
                    pbi = psA.tile([P, TT], _f32, tag="pbi")
                    for di in range(ND):
                        nc.tensor.matmul(pbr[:], brw[kt][di][:], us[di][:],
                                         start=(di == 0), stop=(di == ND - 1))
                        nc.tensor.matmul(pbi[:], biw[kt][di][:], us[di][:],
                                         start=(di == 0), stop=(di == ND - 1))
                    m1 = scr.tile([P, TT], _f32, tag="m1")
                    m2 = scr.tile([P, TT], _f32, tag="m2")
                    nc.vector.tensor_tensor(m1[:], c2[:, sl], pbr[:], _mul)
                    nc.vector.tensor_tensor(m2[:], s2[:, sl], pbi[:], _mul)
                    init_r = 0.0 if tt == 0 else ir[kt][:]
                    nc.vector.tensor_tensor_scan(sr2[:, sl], m1[:], m2[:],
                                                 init_r, _add, _add)
                    m3 = scr.tile([P, TT], _f32, tag="m1")
                    m4 = scr.tile([P, TT], _f32, tag="m2")
                    nc.vector.tensor_tensor(m3[:], c2[:, sl], pbi[:], _mul)
                    nc.vector.tensor_tensor(m4[:], s2[:, sl], pbr[:], _mul)
                    init_i = 0.0 if tt == 0 else ii[kt][:]
                    nc.vector.tensor_tensor_scan(si2[:, sl], m3[:], m4[:],
                                                 init_i, _add, _sub)
                    if tt < NT - 1:
                        e = (h + 1) * TT
                        nc.scalar.copy(ir[kt][:], sr2[:, e - 1:e])
                        nc.scalar.copy(ii[kt][:], si2[:, e - 1:e])

                # untwist on [P, 2*TT]: real part on DVE, imag on GPSIMD
                w1 = scr.tile([P, W], _f32, tag="w1")
                w2 = scr.tile([P, W], _f32, tag="w2")
                hrp = hbuf.tile([P, W], _f32r, name=f"hr2_{p}", tag=f"hr2_{p}")
                hip = hbuf.tile([P, W], _f32r, name=f"hi2_{p}", tag=f"hi2_{p}")
                nc.vector.tensor_tensor(w1[:], c2[:], sr2[:], _mul)
                nc.vector.tensor_tensor(w2[:], s2[:], si2[:], _mul)
                nc.vector.tensor_tensor(hrp[:], w1[:], w2[:], _sub)
                w3 = scr.tile([P, W], _f32, tag="w3", bufs=1)
                w4 = scr.tile([P, W], _f32, tag="w4", bufs=1)
                nc.gpsimd.tensor_tensor(w3[:], c2[:], si2[:], _mul)
                nc.gpsimd.tensor_tensor(w4[:], s2[:], sr2[:], _mul)
                nc.gpsimd.tensor_tensor(hip[:], w3[:], w4[:], _add)
                hr2.append(hrp)
                hi2.append(hip)

                if tt == 0:
                    # A-projection weights: needed first at this tt's MM3;
                    # emitted here so they trail the startup-critical loads
                    for kt in (kt0, kt1):
                        ta = wa.tile([P, DM], _f32r, name=f"art{kt}",
                                     tag=f"art{kt}")
                        nc.sync.dma_start(ta[:], art_d[kt])
                        art[kt] = ta
                        ti = wa.tile([P, DM], _f32r, name=f"ait{kt}",
                                     tag=f"ait{kt}")
                        nc.sync.dma_start(ti[:], ait_d[kt])
                        ait[kt] = ti

            # phase B for the PREVIOUS t-tile (1-iteration software
            # pipeline skew: PE chews on tt-1 outputs while DVE/GPSIMD
            # process tt)
            if tt > 0:
                _emit_mm3(nc, psY, sbuf_s, art, ait, prev_hr2, prev_hi2,
                          yt_d, tt - 1)
            prev_hr2, prev_hi2 = hr2, hi2
        _emit_mm3(nc, psY, sbuf_s, art, ait, prev_hr2, prev_hi2, yt_d, NT - 1)


def _build():
    nc = bacc.Bacc("TRN2", target_bir_lowering=False, debug=False,
                   num_devices=BATCH)
    ut_d = nc.dram_tensor("ut", [ND, P, SEQ], _f32r, kind="ExternalInput")
    brt_d = nc.dram_tensor("brt", [ND, P, K], _f32r, kind="ExternalInput")
    bit_d = nc.dram_tensor("bit", [ND, P, K], _f32r, kind="ExternalInput")
    art_d = nc.dram_tensor("art", [NK, P, DM], _f32r, kind="ExternalInput")
    ait_d = nc.dram_tensor("ait", [NK, P, DM], _f32r, kind="ExternalInput")
    cs_d = nc.dram_tensor("cs", [NK, P, SEQ], _f32, kind="ExternalInput")
    sn_d = nc.dram_tensor("sn", [NK, P, SEQ], _f32, kind="ExternalInput")
    yt_d = nc.dram_tensor("yt", [ND, P, SEQ], _f32, kind="ExternalOutput")
    with tile.TileContext(nc) as tc:
        _emit(tc, nc, (ut_d, brt_d, bit_d, art_d, ait_d, cs_d, sn_d, yt_d))
    nc.compile()
    return nc


def _host_prep(a_params, B_w, C_w):
    """Fold the DFT into B/C and build phase tables (float64 on host)."""
    n, half = NSTATE, K
    a = a_params.astype(np.float64)
    a_full = np.zeros(n)
    a_full[1:half] = a[:half - 1]
    a_full[half + 1:] = -a[:half - 1][::-1]
    omega = np.fft.fft(a_full).imag
    theta = -2.0 * np.arctan(omega)          # lambda_k = e^{i theta_k}

    Bf = np.fft.fft(B_w.astype(np.float64), axis=0)[:half + 1]      # (513, d)
    G = np.conj(np.fft.fft(C_w.astype(np.float64), axis=1))[:, :half + 1]

    Br = np.empty((K, DM))
    Bi = np.empty((K, DM))
    Br[0], Bi[0] = Bf[0].real, Bf[half].real   # DC + Nyquist packed, theta=0
    Br[1:], Bi[1:] = Bf[1:half].real, Bf[1:half].imag
    Ar = np.empty((DM, K))
    Ai = np.empty((DM, K))
    Ar[:, 0] = (1.0 / n) * G[:, 0].real
    Ai[:, 0] = (1.0 / n) * G[:, half].real
    Ar[:, 1:] = (2.0 / n) * G[:, 1:half].real
    Ai[:, 1:] = -(2.0 / n) * G[:, 1:half].imag
    th = theta[:half].copy()
    th[0] = 0.0

    ang = np.outer(th, np.arange(SEQ, dtype=np.float64))   # (K, SEQ)
    f32 = np.float32

    def tiles(m, p):          # (R, C) -> (R//p, p, C) contiguous f32
        return np.ascontiguousarray(m.reshape(m.shape[0] // p, p, m.shape[1]),
                                    dtype=f32)

    return {
        "brt": tiles(Br.T.copy(), P),       # (8, 128, 512)  BrT[d, k]
        "bit": tiles(Bi.T.copy(), P),
        "art": tiles(Ar.T.copy(), P),       # (4, 128, 1024) ArT[k, d]
        "ait": tiles(Ai.T.copy(), P),
        "cs": tiles(np.cos(ang), P),        # (4, 128, 2048)
        "sn": tiles(np.sin(ang), P),
    }


def _run(u, a_params, B_w, C_w, D, trace=False):
    global _COMPILED
    if _COMPILED is None:
        _COMPILED = _build()
    nc = _COMPILED
    shared = _host_prep(np.asarray(a_params), np.asarray(B_w), np.asarray(C_w))
    u = np.asarray(u)
    in_maps = []
    for b in range(BATCH):
        m = dict(shared)
        m["ut"] = np.ascontiguousarray(
            u[b].T.reshape(ND, P, SEQ), dtype=np.float32)
        in_maps.append(m)
    res = run_bass_kernel_spmd(nc, in_maps, core_ids=list(range(BATCH)),
                               trace=trace)
    y = np.empty((BATCH, SEQ, DM), dtype=np.float32)
    for b in range(BATCH):
        y[b] = res.results[b]["yt"].reshape(DM, SEQ).T
    y += np.asarray(D)[None, None, :] * u
    return y, res


def kernel(u, a_params, B_w, C_w, D):
    y, _ = _run(u, a_params, B_w, C_w, D)
    return y


# revision 14
# speedup vs baseline: 1.4101x; 1.2501x over previous
"""Cayley-circulant SSM layer as a Trainium2 Bass kernel.

Math: h_t = W h_{t-1} + B u_t, y_t = C h_t + D u_t, where W is a real
orthogonal circulant (Cayley transform of a skew-circulant) diagonalized
by the DFT with unit-modulus eigenvalues lambda_k = e^{i theta_k}.

Device algorithm (frequency-domain associative scan):
  1. Fold the rfft into B and C on the host (weight preprocessing):
     buhat_t = (F B) u_t restricted to 512 packed real frequency
     channels (Hermitian symmetry; DC and Nyquist share channel 0 as
     (re, im) with theta=0).
  2. The recurrence hhat_t = lambda * hhat_{t-1} + buhat_t becomes,
     with z_t = conj(lambda)^t * buhat_t, a plain cumulative sum:
     hhat_t = lambda^t * cumsum(z)_t.  |lambda|=1 so this is exact.
  3. y_t = Re(G hhat_t) = Ar @ hhat_r + Ai @ hhat_i + D u_t.

Per-core layout (data-parallel over batch, 1 row per NeuronCore):
  MM1  (PE, fp32r):  bu_{r,i}(k,t) = BrT/BiT.T @ uT          (d contracted)
  twist (DVE):       m1 = c*bur, m2 = s*bui, m3 = c*bui, m4 = s*bur
  scan  (DVE):       Sr = cumsum(m1+m2), Si = cumsum(m3-m4)   (fused)
  untwist (DVE):     hr = c*Sr - s*Si, hi = c*Si + s*Sr       (fp32r out)
  MM3  (PE, fp32r):  yT(d,t) = ArT.T @ hr + AiT.T @ hi        (k contracted)
cos/sin tables are host-precomputed in float64 per (k, t).
"""

import numpy as np

import concourse.bass as bass  # noqa: F401  (registers engine types)
import concourse.mybir as mybir
import concourse.tile as tile
from concourse import bacc
from concourse.bass_utils import run_bass_kernel_spmd

BATCH, SEQ, DM, NSTATE = 8, 2048, 1024, 1024
K = NSTATE // 2          # packed real frequency channels
P = 128                  # partitions
TT = 512                 # t-tile width (one PSUM bank of fp32)
ND, NK, NT = DM // P, K // P, SEQ // TT

_f32 = mybir.dt.float32
_f32r = mybir.dt.float32r
_add = mybir.AluOpType.add
_sub = mybir.AluOpType.subtract
_mul = mybir.AluOpType.mult

_COMPILED = None


def _emit_mm3(nc, psY, sbuf_s, art, ait, hr2, hi2, yt_d, tt, di0, di1):
    ts = slice(tt * TT, (tt + 1) * TT)
    NP = NK // 2
    for di in range(di0, di1):
        py = psY.tile([P, TT], _f32, tag="py")
        for p in range(NP):
            for h, kt in ((0, 2 * p), (1, 2 * p + 1)):
                sl = slice(h * TT, (h + 1) * TT)
                nc.tensor.matmul(py[:], art[kt][:, di * P:(di + 1) * P],
                                 hr2[p][:, sl],
                                 start=(p == 0 and h == 0), stop=False)
                nc.tensor.matmul(py[:], ait[kt][:, di * P:(di + 1) * P],
                                 hi2[p][:, sl], start=False,
                                 stop=(p == NP - 1 and h == 1))
        ysb = sbuf_s.tile([P, TT], _f32, tag="ysb", bufs=2)
        nc.scalar.copy(ysb[:], py[:])
        nc.sync.dma_start(yt_d[tt, di], ysb[:])


def _emit(tc, nc, dr):
    ut_d, brt_d, bit_d, art_d, ait_d, cs_d, sn_d, yt_d = dr
    NP = NK // 2      # k-tile pairs: SBUF elementwise ops run on [P, 2*TT]
    W = 2 * TT
    with (
        tc.tile_pool(name="ust", bufs=2) as ust,
        tc.tile_pool(name="wb", bufs=1) as wb,
        tc.tile_pool(name="wa", bufs=1) as wa,
        tc.tile_pool(name="tbl", bufs=3) as tbl,
        tc.tile_pool(name="hbuf", bufs=2) as hbuf,
        tc.tile_pool(name="scr", bufs=2) as scr,
        tc.tile_pool(name="sbuf_s", bufs=3) as sbuf_s,
        tc.tile_pool(name="psA", bufs=2, space="PSUM") as psA,
        tc.tile_pool(name="psY", bufs=4, space="PSUM") as psY,
    ):
        art = [None] * NK
        ait = [None] * NK
        brw = [None] * NK
        biw = [None] * NK
        prev_s = {}   # (p) -> (sr2, si2) of previous t-tile

        for tt in range(NT):
            ts = slice(tt * TT, (tt + 1) * TT)
            us = []
            for di in range(ND):
                t = ust.tile([P, TT], _f32r, tag=f"us{di}")
                nc.sync.dma_start(t[:], ut_d[tt, di])
                us.append(t)
            hr2, hi2 = [], []
            for p in range(NP):
                kt0, kt1 = 2 * p, 2 * p + 1
                c2 = tbl.tile([P, W], _f32, tag="c2")
                s2 = tbl.tile([P, W], _f32, tag="s2")
                nc.sync.dma_start(c2[:, :TT], cs_d[tt, kt0])
                nc.sync.dma_start(c2[:, TT:], cs_d[tt, kt1])
                nc.sync.dma_start(s2[:, :TT], sn_d[tt, kt0])
                nc.sync.dma_start(s2[:, TT:], sn_d[tt, kt1])

                sr2 = sbuf_s.tile([P, W], _f32, tag="sr2")
                si2 = sbuf_s.tile([P, W], _f32, tag="si2")
                for h, kt in ((0, kt0), (1, kt1)):
                    sl = slice(h * TT, (h + 1) * TT)
                    if tt == 0:
                        tb = wb.tile([P, DM], _f32r, name=f"brw{kt}",
                                     tag=f"brw{kt}")
                        nc.sync.dma_start(tb[:], brt_d[kt])
                        brw[kt] = tb
                        ti = wb.tile([P, DM], _f32r, name=f"biw{kt}",
                                     tag=f"biw{kt}")
                        nc.sync.dma_start(ti[:], bit_d[kt])
                        biw[kt] = ti
                    pbr = psA.tile([P, TT], _f32, tag="pbr")
                    pbi = psA.tile([P, TT], _f32, tag="pbi")
                    for di in range(ND):
                        dsl = slice(di * P, (di + 1) * P)
                        nc.tensor.matmul(pbr[:], brw[kt][:, dsl], us[di][:],
                                         start=(di == 0), stop=(di == ND - 1))
                        nc.tensor.matmul(pbi[:], biw[kt][:, dsl], us[di][:],
                                         start=(di == 0), stop=(di == ND - 1))
                    m1 = scr.tile([P, TT], _f32, tag="m1")
                    m2 = scr.tile([P, TT], _f32, tag="m2")
                    nc.vector.tensor_tensor(m1[:], c2[:, sl], pbr[:], _mul)
                    nc.vector.tensor_tensor(m2[:], s2[:, sl], pbi[:], _mul)
                    e = (h + 1) * TT
                    init_r = 0.0 if tt == 0 else prev_s[p][0][:, e - 1:e]
                    nc.vector.tensor_tensor_scan(sr2[:, sl], m1[:], m2[:],
                                                 init_r, _add, _add)
                    m3 = scr.tile([P, TT], _f32, tag="m1")
                    m4 = scr.tile([P, TT], _f32, tag="m2")
                    nc.vector.tensor_tensor(m3[:], c2[:, sl], pbi[:], _mul)
                    nc.vector.tensor_tensor(m4[:], s2[:, sl], pbr[:], _mul)
                    init_i = 0.0 if tt == 0 else prev_s[p][1][:, e - 1:e]
                    nc.vector.tensor_tensor_scan(si2[:, sl], m3[:], m4[:],
                                                 init_i, _add, _sub)

                # untwist on [P, 2*TT]: real part on DVE, imag on GPSIMD
                w1 = scr.tile([P, W], _f32, tag="w1", bufs=1)
                w2 = scr.tile([P, W], _f32, tag="w2", bufs=1)
                hrp = hbuf.tile([P, W], _f32r, name=f"hr2_{p}", tag=f"hr2_{p}")
                hip = hbuf.tile([P, W], _f32r, name=f"hi2_{p}", tag=f"hi2_{p}")
                nc.vector.tensor_tensor(w1[:], c2[:], sr2[:], _mul)
                nc.vector.tensor_tensor(w2[:], s2[:], si2[:], _mul)
                nc.vector.tensor_tensor(hrp[:], w1[:], w2[:], _sub)
                w3 = scr.tile([P, W], _f32, tag="w3", bufs=1)
                w4 = scr.tile([P, W], _f32, tag="w4", bufs=1)
                nc.vector.tensor_tensor(w3[:], c2[:], si2[:], _mul)
                nc.vector.tensor_tensor(w4[:], s2[:], sr2[:], _mul)
                nc.gpsimd.tensor_tensor(hip[:], w3[:], w4[:], _add)
                hr2.append(hrp)
                hi2.append(hip)
                prev_s[p] = (sr2, si2)
                if tt > 0:
                    half = ND // NP
                    _emit_mm3(nc, psY, sbuf_s, art, ait, prev_hr2, prev_hi2,
                              yt_d, tt - 1, p * half, (p + 1) * half)


            if tt == 0:
                # A-projection weights: first needed by the skewed MM3 in
                # iteration 1; emitted last so they trail the
                # startup-critical u/B/table loads on the DMA queues
                for kt in range(NK):
                    ta = wa.tile([P, DM], _f32r, name=f"art{kt}",
                                 tag=f"art{kt}")
                    nc.sync.dma_start(ta[:], art_d[kt])
                    art[kt] = ta
                    ti = wa.tile([P, DM], _f32r, name=f"ait{kt}",
                                 tag=f"ait{kt}")
                    nc.sync.dma_start(ti[:], ait_d[kt])
                    ait[kt] = ti
            prev_hr2, prev_hi2 = hr2, hi2
        _emit_mm3(nc, psY, sbuf_s, art, ait, prev_hr2, prev_hi2, yt_d,
                  NT - 1, 0, ND)


def _build():
    nc = bacc.Bacc("TRN2", target_bir_lowering=False, debug=False,
                   num_devices=BATCH)
    ut_d = nc.dram_tensor("ut", [NT, ND, P, TT], _f32r, kind="ExternalInput")
    brt_d = nc.dram_tensor("brt", [NK, P, DM], _f32r, kind="ExternalInput")
    bit_d = nc.dram_tensor("bit", [NK, P, DM], _f32r, kind="ExternalInput")
    art_d = nc.dram_tensor("art", [NK, P, DM], _f32r, kind="ExternalInput")
    ait_d = nc.dram_tensor("ait", [NK, P, DM], _f32r, kind="ExternalInput")
    cs_d = nc.dram_tensor("cs", [NT, NK, P, TT], _f32, kind="ExternalInput")
    sn_d = nc.dram_tensor("sn", [NT, NK, P, TT], _f32, kind="ExternalInput")
    yt_d = nc.dram_tensor("yt", [NT, ND, P, TT], _f32, kind="ExternalOutput")
    with tile.TileContext(nc) as tc:
        _emit(tc, nc, (ut_d, brt_d, bit_d, art_d, ait_d, cs_d, sn_d, yt_d))
    nc.compile()
    return nc


def _host_prep(a_params, B_w, C_w):
    """Fold the DFT into B/C and build phase tables (float64 on host)."""
    n, half = NSTATE, K
    a = a_params.astype(np.float64)
    a_full = np.zeros(n)
    a_full[1:half] = a[:half - 1]
    a_full[half + 1:] = -a[:half - 1][::-1]
    omega = np.fft.fft(a_full).imag
    theta = -2.0 * np.arctan(omega)          # lambda_k = e^{i theta_k}

    Bf = np.fft.fft(B_w.astype(np.float64), axis=0)[:half + 1]      # (513, d)
    G = np.conj(np.fft.fft(C_w.astype(np.float64), axis=1))[:, :half + 1]

    Br = np.empty((K, DM))
    Bi = np.empty((K, DM))
    Br[0], Bi[0] = Bf[0].real, Bf[half].real   # DC + Nyquist packed, theta=0
    Br[1:], Bi[1:] = Bf[1:half].real, Bf[1:half].imag
    Ar = np.empty((DM, K))
    Ai = np.empty((DM, K))
    Ar[:, 0] = (1.0 / n) * G[:, 0].real
    Ai[:, 0] = (1.0 / n) * G[:, half].real
    Ar[:, 1:] = (2.0 / n) * G[:, 1:half].real
    Ai[:, 1:] = -(2.0 / n) * G[:, 1:half].imag
    th = theta[:half].copy()
    th[0] = 0.0

    ang = np.outer(th, np.arange(SEQ, dtype=np.float64))   # (K, SEQ)
    f32 = np.float32

    def tiles_tt(m, p):       # (R, SEQ) -> (NT, R//p, p, TT) contiguous
        r = m.reshape(m.shape[0] // p, p, NT, TT)
        return np.ascontiguousarray(r.transpose(2, 0, 1, 3), dtype=f32)

    def tiles(m, p):          # (R, C) -> (R//p, p, C) contiguous f32
        return np.ascontiguousarray(m.reshape(m.shape[0] // p, p, m.shape[1]),
                                    dtype=f32)

    # packed B layout: [kt, p, di*128+j] = Br[kt*128+j, di*128+p]
    def bpack(M):                 # (K, DM) -> (NK, P, DM)
        r = M.reshape(NK, P, ND, P)          # [kt, j, di, p]
        return np.ascontiguousarray(r.transpose(0, 3, 2, 1).reshape(NK, P, DM),
                                    dtype=np.float32)

    return {
        "brt": bpack(Br),                   # (4, 128, 1024)
        "bit": bpack(Bi),
        "art": tiles(Ar.T.copy(), P),       # (4, 128, 1024) ArT[k, d]
        "ait": tiles(Ai.T.copy(), P),
        "cs": tiles_tt(np.cos(ang), P),     # (NT, NK, 128, TT)
        "sn": tiles_tt(np.sin(ang), P),
    }


def _run(u, a_params, B_w, C_w, D, trace=False):
    global _COMPILED
    if _COMPILED is None:
        _COMPILED = _build()
    nc = _COMPILED
    shared = _host_prep(np.asarray(a_params), np.asarray(B_w), np.asarray(C_w))
    u = np.asarray(u)
    in_maps = []
    for b in range(BATCH):
        m = dict(shared)
        m["ut"] = np.ascontiguousarray(
            u[b].T.reshape(ND, P, NT, TT).transpose(2, 0, 1, 3),
            dtype=np.float32)
        in_maps.append(m)
    res = run_bass_kernel_spmd(nc, in_maps, core_ids=list(range(BATCH)),
                               trace=trace)
    y = np.empty((BATCH, SEQ, DM), dtype=np.float32)
    for b in range(BATCH):
        yt = res.results[b]["yt"]            # (NT, ND, P, TT)
        y[b] = yt.transpose(1, 2, 0, 3).reshape(DM, SEQ).T
    y += np.asarray(D)[None, None, :] * u
    return y, res


def kernel(u, a_params, B_w, C_w, D):
    y, _ = _run(u, a_params, B_w, C_w, D)
    return y


# revision 15
# speedup vs baseline: 1.7319x; 1.2282x over previous
"""Cayley-circulant SSM layer as a Trainium2 Bass kernel.

Math: h_t = W h_{t-1} + B u_t, y_t = C h_t + D u_t, where W is a real
orthogonal circulant (Cayley transform of a skew-circulant) diagonalized
by the DFT with unit-modulus eigenvalues lambda_k = e^{i theta_k}.

Device algorithm (frequency-domain associative scan):
  1. Fold the rfft into B and C on the host (weight preprocessing):
     buhat_t = (F B) u_t restricted to 512 packed real frequency
     channels (Hermitian symmetry; DC and Nyquist share channel 0 as
     (re, im) with theta=0).
  2. The recurrence hhat_t = lambda * hhat_{t-1} + buhat_t becomes,
     with z_t = conj(lambda)^t * buhat_t, a plain cumulative sum:
     hhat_t = lambda^t * cumsum(z)_t.  |lambda|=1 so this is exact.
  3. y_t = Re(G hhat_t) = Ar @ hhat_r + Ai @ hhat_i + D u_t.

Per-core layout (data-parallel over batch, 1 row per NeuronCore):
  MM1  (PE, fp32r):  bu_{r,i}(k,t) = BrT/BiT.T @ uT          (d contracted)
  twist (DVE):       m1 = c*bur, m2 = s*bui, m3 = c*bui, m4 = s*bur
  scan  (DVE):       Sr = cumsum(m1+m2), Si = cumsum(m3-m4)   (fused)
  untwist (DVE):     hr = c*Sr - s*Si, hi = c*Si + s*Sr       (fp32r out)
  MM3  (PE, fp32r):  yT(d,t) = ArT.T @ hr + AiT.T @ hi        (k contracted)
cos/sin tables are host-precomputed in float64 per (k, t).
"""

import numpy as np

import concourse.bass as bass  # noqa: F401  (registers engine types)
import concourse.mybir as mybir
import concourse.tile as tile
from concourse import bacc
from concourse.bass_utils import run_bass_kernel_spmd

BATCH, SEQ, DM, NSTATE = 8, 2048, 1024, 1024
K = NSTATE // 2          # packed real frequency channels
P = 128                  # partitions
TT = 512                 # t-tile width (one PSUM bank of fp32)
ND, NK, NT = DM // P, K // P, SEQ // TT

_f32 = mybir.dt.float32
_f32r = mybir.dt.float32r
_f16 = mybir.dt.float16
_add = mybir.AluOpType.add
_sub = mybir.AluOpType.subtract
_mul = mybir.AluOpType.mult

_COMPILED = None


def _emit_mm3(nc, psY, sbuf_s, art, ait, hr2, hi2, yt_d, tt, di0, di1):
    ts = slice(tt * TT, (tt + 1) * TT)
    NP = NK // 2
    for di in range(di0, di1):
        py = psY.tile([P, TT], _f32, tag="py")
        for p in range(NP):
            for h, kt in ((0, 2 * p), (1, 2 * p + 1)):
                sl = slice(h * TT, (h + 1) * TT)
                nc.tensor.matmul(py[:], art[kt][:, di * P:(di + 1) * P],
                                 hr2[p][:, sl],
                                 start=(p == 0 and h == 0), stop=False)
                nc.tensor.matmul(py[:], ait[kt][:, di * P:(di + 1) * P],
                                 hi2[p][:, sl], start=False,
                                 stop=(p == NP - 1 and h == 1))
        ysb = sbuf_s.tile([P, TT], _f32, tag="ysb", bufs=2)
        nc.scalar.copy(ysb[:], py[:])
        nc.sync.dma_start(yt_d[tt, di], ysb[:])


def _emit(tc, nc, dr):
    ut_d, brt_d, bit_d, art_d, ait_d, cs_d, sn_d, yt_d = dr
    NP = NK // 2      # k-tile pairs: SBUF elementwise ops run on [P, 2*TT]
    W = 2 * TT
    with (
        tc.tile_pool(name="ust", bufs=2) as ust,
        tc.tile_pool(name="wb", bufs=1) as wb,
        tc.tile_pool(name="wa", bufs=1) as wa,
        tc.tile_pool(name="tbl", bufs=3) as tbl,
        tc.tile_pool(name="hbuf", bufs=2) as hbuf,
        tc.tile_pool(name="scr", bufs=2) as scr,
        tc.tile_pool(name="sbuf_s", bufs=3) as sbuf_s,
        tc.tile_pool(name="psA", bufs=2, space="PSUM") as psA,
        tc.tile_pool(name="psY", bufs=4, space="PSUM") as psY,
    ):
        art = [None] * NK
        ait = [None] * NK
        brw = [None] * NK
        biw = [None] * NK
        prev_s = {}   # (p) -> (sr2, si2) of previous t-tile

        # HAM warmup: dummy matmuls while the startup DMAs are in flight so
        # the PE clock is at 8/8 when the first real MM1 issues (cold MMs
        # run at half rate for the first ~3.4us of activity otherwise)
        warm_w = scr.tile([P, TT], _f32, name="warm_w", tag="warm_w", bufs=1)
        warm_ps = psY.tile([P, TT], _f32, tag="py")
        nc.vector.memset(warm_w[:], 0.0)
        for _ in range(12):
            nc.tensor.matmul(warm_ps[:, :256], warm_w[:, :P],
                             warm_w[:, :256], start=True, stop=True)

        for tt in range(NT):
            if tt == 1:
                # A-projection weights: first needed by the skewed MM3
                # later this iteration; loading now keeps the tt=0 ramp
                # DMA queues clear for the startup-critical u/B/tables
                for kt in range(NK):
                    ta = wa.tile([P, DM], _f16, name=f"art{kt}",
                                 tag=f"art{kt}")
                    nc.sync.dma_start(ta[:], art_d[kt])
                    art[kt] = ta
                    ti = wa.tile([P, DM], _f16, name=f"ait{kt}",
                                 tag=f"ait{kt}")
                    nc.sync.dma_start(ti[:], ait_d[kt])
                    ait[kt] = ti
            ts = slice(tt * TT, (tt + 1) * TT)
            us = []
            for di in range(ND):
                t = ust.tile([P, TT], _f16, tag=f"us{di}")
                nc.sync.dma_start(t[:], ut_d[tt, di])
                us.append(t)
            hr2, hi2 = [], []
            for p in range(NP):
                kt0, kt1 = 2 * p, 2 * p + 1
                if tt == 0:
                    for kt in (kt0, kt1):
                        tb = wb.tile([P, DM], _f16, name=f"brw{kt}",
                                     tag=f"brw{kt}")
                        nc.sync.dma_start(tb[:], brt_d[kt])
                        brw[kt] = tb
                        ti = wb.tile([P, DM], _f16, name=f"biw{kt}",
                                     tag=f"biw{kt}")
                        nc.sync.dma_start(ti[:], bit_d[kt])
                        biw[kt] = ti
                c2 = tbl.tile([P, W], _f32, tag="c2")
                s2 = tbl.tile([P, W], _f32, tag="s2")
                nc.sync.dma_start(c2[:, :TT], cs_d[tt, kt0])
                nc.sync.dma_start(c2[:, TT:], cs_d[tt, kt1])
                nc.sync.dma_start(s2[:, :TT], sn_d[tt, kt0])
                nc.sync.dma_start(s2[:, TT:], sn_d[tt, kt1])

                sr2 = sbuf_s.tile([P, W], _f16, tag="sr2")
                si2 = sbuf_s.tile([P, W], _f16, tag="si2")
                for h, kt in ((0, kt0), (1, kt1)):
                    sl = slice(h * TT, (h + 1) * TT)
                    pbr = psA.tile([P, TT], _f32, tag="pbr")
                    pbi = psA.tile([P, TT], _f32, tag="pbi")
                    for di in range(ND):
                        dsl = slice(di * P, (di + 1) * P)
                        nc.tensor.matmul(pbr[:], brw[kt][:, dsl], us[di][:],
                                         start=(di == 0), stop=(di == ND - 1))
                        nc.tensor.matmul(pbi[:], biw[kt][:, dsl], us[di][:],
                                         start=(di == 0), stop=(di == ND - 1))
                    m1 = scr.tile([P, TT], _f32, tag="m1")
                    m2 = scr.tile([P, TT], _f32, tag="m2")
                    nc.vector.tensor_tensor(m1[:], c2[:, sl], pbr[:], _mul)
                    nc.vector.tensor_tensor(m2[:], s2[:, sl], pbi[:], _mul)
                    e = (h + 1) * TT
                    init_r = 0.0 if tt == 0 else prev_s[p][0][:, e - 1:e]
                    nc.vector.tensor_tensor_scan(sr2[:, sl], m1[:], m2[:],
                                                 init_r, _add, _add)
                    m3 = scr.tile([P, TT], _f32, tag="m1")
                    m4 = scr.tile([P, TT], _f32, tag="m2")
                    nc.vector.tensor_tensor(m3[:], c2[:, sl], pbi[:], _mul)
                    nc.vector.tensor_tensor(m4[:], s2[:, sl], pbr[:], _mul)
                    init_i = 0.0 if tt == 0 else prev_s[p][1][:, e - 1:e]
                    nc.vector.tensor_tensor_scan(si2[:, sl], m3[:], m4[:],
                                                 init_i, _add, _sub)

                # untwist on [P, 2*TT] in fp16 (DVE 2x mode); fp16
                # table copies generated on the otherwise-idle ACT engine
                c2h = scr.tile([P, W], _f16, tag="c2h", bufs=2)
                s2h = scr.tile([P, W], _f16, tag="s2h", bufs=2)
                nc.scalar.copy(c2h[:], c2[:])
                nc.scalar.copy(s2h[:], s2[:])
                w1 = scr.tile([P, W], _f16, tag="w1", bufs=1)
                w2 = scr.tile([P, W], _f16, tag="w2", bufs=1)
                hrp = hbuf.tile([P, W], _f16, name=f"hr2_{p}", tag=f"hr2_{p}")
                hip = hbuf.tile([P, W], _f16, name=f"hi2_{p}", tag=f"hi2_{p}")
                nc.vector.tensor_tensor(w1[:], c2h[:], sr2[:], _mul)
                nc.vector.tensor_tensor(w2[:], s2h[:], si2[:], _mul)
                nc.vector.tensor_tensor(hrp[:], w1[:], w2[:], _sub)
                w3 = scr.tile([P, W], _f16, tag="w3", bufs=1)
                w4 = scr.tile([P, W], _f16, tag="w4", bufs=1)
                nc.vector.tensor_tensor(w3[:], c2h[:], si2[:], _mul)
                nc.vector.tensor_tensor(w4[:], s2h[:], sr2[:], _mul)
                if tt == NT - 1:
                    nc.vector.tensor_tensor(hip[:], w3[:], w4[:], _add)
                else:
                    nc.gpsimd.tensor_tensor(hip[:], w3[:], w4[:], _add)
                hr2.append(hrp)
                hi2.append(hip)
                prev_s[p] = (sr2, si2)
                if tt > 0:
                    half = ND // NP
                    _emit_mm3(nc, psY, sbuf_s, art, ait, prev_hr2, prev_hi2,
                              yt_d, tt - 1, p * half, (p + 1) * half)


            prev_hr2, prev_hi2 = hr2, hi2
        _emit_mm3(nc, psY, sbuf_s, art, ait, prev_hr2, prev_hi2, yt_d,
                  NT - 1, 0, ND)


def _build():
    nc = bacc.Bacc("TRN2", target_bir_lowering=False, debug=False,
                   num_devices=BATCH)
    ut_d = nc.dram_tensor("ut", [NT, ND, P, TT], _f16, kind="ExternalInput")
    brt_d = nc.dram_tensor("brt", [NK, P, DM], _f16, kind="ExternalInput")
    bit_d = nc.dram_tensor("bit", [NK, P, DM], _f16, kind="ExternalInput")
    art_d = nc.dram_tensor("art", [NK, P, DM], _f16, kind="ExternalInput")
    ait_d = nc.dram_tensor("ait", [NK, P, DM], _f16, kind="ExternalInput")
    cs_d = nc.dram_tensor("cs", [NT, NK, P, TT], _f32, kind="ExternalInput")
    sn_d = nc.dram_tensor("sn", [NT, NK, P, TT], _f32, kind="ExternalInput")
    yt_d = nc.dram_tensor("yt", [NT, ND, P, TT], _f32, kind="ExternalOutput")
    with tile.TileContext(nc) as tc:
        _emit(tc, nc, (ut_d, brt_d, bit_d, art_d, ait_d, cs_d, sn_d, yt_d))
    nc.compile()
    return nc


def _host_prep(a_params, B_w, C_w):
    """Fold the DFT into B/C and build phase tables (float64 on host)."""
    n, half = NSTATE, K
    a = a_params.astype(np.float64)
    a_full = np.zeros(n)
    a_full[1:half] = a[:half - 1]
    a_full[half + 1:] = -a[:half - 1][::-1]
    omega = np.fft.fft(a_full).imag
    theta = -2.0 * np.arctan(omega)          # lambda_k = e^{i theta_k}

    Bf = np.fft.fft(B_w.astype(np.float64), axis=0)[:half + 1]      # (513, d)
    G = np.conj(np.fft.fft(C_w.astype(np.float64), axis=1))[:, :half + 1]

    Br = np.empty((K, DM))
    Bi = np.empty((K, DM))
    Br[0], Bi[0] = Bf[0].real, Bf[half].real   # DC + Nyquist packed, theta=0
    Br[1:], Bi[1:] = Bf[1:half].real, Bf[1:half].imag
    Ar = np.empty((DM, K))
    Ai = np.empty((DM, K))
    Ar[:, 0] = (1.0 / n) * G[:, 0].real
    Ai[:, 0] = (1.0 / n) * G[:, half].real
    Ar[:, 1:] = (2.0 / n) * G[:, 1:half].real
    Ai[:, 1:] = -(2.0 / n) * G[:, 1:half].imag
    th = theta[:half].copy()
    th[0] = 0.0

    ang = np.outer(th, np.arange(SEQ, dtype=np.float64))   # (K, SEQ)
    f32 = np.float32

    def tiles_tt(m, p):       # (R, SEQ) -> (NT, R//p, p, TT) contiguous
        r = m.reshape(m.shape[0] // p, p, NT, TT)
        return np.ascontiguousarray(r.transpose(2, 0, 1, 3), dtype=f32)

    def tiles(m, p):          # (R, C) -> (R//p, p, C) contiguous f32
        return np.ascontiguousarray(m.reshape(m.shape[0] // p, p, m.shape[1]),
                                    dtype=f32)

    # packed B layout: [kt, p, di*128+j] = Br[kt*128+j, di*128+p]
    def bpack(M):                 # (K, DM) -> (NK, P, DM)
        r = M.reshape(NK, P, ND, P)          # [kt, j, di, p]
        return np.ascontiguousarray(r.transpose(0, 3, 2, 1).reshape(NK, P, DM),
                                    dtype=np.float32)

    return {
        "brt": bpack(Br).astype(np.float16),
        "bit": bpack(Bi).astype(np.float16),
        "art": tiles(Ar.T.copy(), P).astype(np.float16),
        "ait": tiles(Ai.T.copy(), P).astype(np.float16),
        "cs": tiles_tt(np.cos(ang), P),     # (NT, NK, 128, TT)
        "sn": tiles_tt(np.sin(ang), P),
    }


def _run(u, a_params, B_w, C_w, D, trace=False):
    global _COMPILED
    if _COMPILED is None:
        _COMPILED = _build()
    nc = _COMPILED
    shared = _host_prep(np.asarray(a_params), np.asarray(B_w), np.asarray(C_w))
    u = np.asarray(u)
    in_maps = []
    for b in range(BATCH):
        m = dict(shared)
        m["ut"] = np.ascontiguousarray(
            u[b].T.reshape(ND, P, NT, TT).transpose(2, 0, 1, 3),
            dtype=np.float16)
        in_maps.append(m)
    res = run_bass_kernel_spmd(nc, in_maps, core_ids=list(range(BATCH)),
                               trace=trace)
    y = np.empty((BATCH, SEQ, DM), dtype=np.float32)
    for b in range(BATCH):
        yt = res.results[b]["yt"]            # (NT, ND, P, TT)
        y[b] = yt.transpose(1, 2, 0, 3).reshape(DM, SEQ).T
    y += np.asarray(D)[None, None, :] * u
    return y, res


def kernel(u, a_params, B_w, C_w, D):
    y, _ = _run(u, a_params, B_w, C_w, D)
    return y


# revision 16
# speedup vs baseline: 1.7356x; 1.0021x over previous
"""Cayley-circulant SSM layer as a Trainium2 Bass kernel.

Math: h_t = W h_{t-1} + B u_t, y_t = C h_t + D u_t, where W is a real
orthogonal circulant (Cayley transform of a skew-circulant) diagonalized
by the DFT with unit-modulus eigenvalues lambda_k = e^{i theta_k}.

Device algorithm (frequency-domain associative scan):
  1. Fold the rfft into B and C on the host (weight preprocessing):
     buhat_t = (F B) u_t restricted to 512 packed real frequency
     channels (Hermitian symmetry; DC and Nyquist share channel 0 as
     (re, im) with theta=0).
  2. The recurrence hhat_t = lambda * hhat_{t-1} + buhat_t becomes,
     with z_t = conj(lambda)^t * buhat_t, a plain cumulative sum:
     hhat_t = lambda^t * cumsum(z)_t.  |lambda|=1 so this is exact.
  3. y_t = Re(G hhat_t) = Ar @ hhat_r + Ai @ hhat_i + D u_t.

Per-core layout (data-parallel over batch, 1 row per NeuronCore):
  MM1  (PE, fp32r):  bu_{r,i}(k,t) = BrT/BiT.T @ uT          (d contracted)
  twist (DVE):       m1 = c*bur, m2 = s*bui, m3 = c*bui, m4 = s*bur
  scan  (DVE):       Sr = cumsum(m1+m2), Si = cumsum(m3-m4)   (fused)
  untwist (DVE):     hr = c*Sr - s*Si, hi = c*Si + s*Sr       (fp32r out)
  MM3  (PE, fp32r):  yT(d,t) = ArT.T @ hr + AiT.T @ hi        (k contracted)
cos/sin tables are host-precomputed in float64 per (k, t).
"""

import numpy as np

import concourse.bass as bass  # noqa: F401  (registers engine types)
import concourse.mybir as mybir
import concourse.tile as tile
from concourse import bacc
from concourse.bass_utils import run_bass_kernel_spmd

BATCH, SEQ, DM, NSTATE = 8, 2048, 1024, 1024
K = NSTATE // 2          # packed real frequency channels
P = 128                  # partitions
TT = 512                 # t-tile width (one PSUM bank of fp32)
ND, NK, NT = DM // P, K // P, SEQ // TT

_f32 = mybir.dt.float32
_f32r = mybir.dt.float32r
_f16 = mybir.dt.float16
_add = mybir.AluOpType.add
_sub = mybir.AluOpType.subtract
_mul = mybir.AluOpType.mult

_COMPILED = None


def _emit_mm3(nc, psY, sbuf_s, art, ait, hr2, hi2, yt_d, tt, di0, di1):
    ts = slice(tt * TT, (tt + 1) * TT)
    NP = NK // 2
    for di in range(di0, di1):
        py = psY.tile([P, TT], _f32, tag="py")
        for p in range(NP):
            for h, kt in ((0, 2 * p), (1, 2 * p + 1)):
                sl = slice(h * TT, (h + 1) * TT)
                nc.tensor.matmul(py[:], art[kt][:, di * P:(di + 1) * P],
                                 hr2[p][:, sl],
                                 start=(p == 0 and h == 0), stop=False)
                nc.tensor.matmul(py[:], ait[kt][:, di * P:(di + 1) * P],
                                 hi2[p][:, sl], start=False,
                                 stop=(p == NP - 1 and h == 1))
        ysb = sbuf_s.tile([P, TT], _f32, tag="ysb", bufs=2)
        nc.scalar.copy(ysb[:], py[:])
        nc.sync.dma_start(yt_d[tt, di], ysb[:])


def _emit(tc, nc, dr):
    ut_d, brt_d, bit_d, art_d, ait_d, cs_d, sn_d, yt_d = dr
    NP = NK // 2      # k-tile pairs: SBUF elementwise ops run on [P, 2*TT]
    W = 2 * TT
    with (
        tc.tile_pool(name="ust", bufs=3) as ust,
        tc.tile_pool(name="wb", bufs=1) as wb,
        tc.tile_pool(name="wa", bufs=1) as wa,
        tc.tile_pool(name="tbl", bufs=3) as tbl,
        tc.tile_pool(name="hbuf", bufs=2) as hbuf,
        tc.tile_pool(name="scr", bufs=2) as scr,
        tc.tile_pool(name="sbuf_s", bufs=3) as sbuf_s,
        tc.tile_pool(name="psA", bufs=2, space="PSUM") as psA,
        tc.tile_pool(name="psY", bufs=4, space="PSUM") as psY,
    ):
        art = [None] * NK
        ait = [None] * NK
        brw = [None] * NK
        biw = [None] * NK
        prev_s = {}   # (p) -> (sr2, si2) of previous t-tile

        # HAM warmup: dummy matmuls while the startup DMAs are in flight so
        # the PE clock is at 8/8 when the first real MM1 issues (cold MMs
        # run at half rate for the first ~3.4us of activity otherwise)
        warm_w = scr.tile([P, TT], _f32, name="warm_w", tag="warm_w", bufs=1)
        warm_ps = psY.tile([P, TT], _f32, tag="py")
        nc.vector.memset(warm_w[:], 0.0)
        for _ in range(12):
            nc.tensor.matmul(warm_ps[:, :256], warm_w[:, :P],
                             warm_w[:, :256], start=True, stop=True)

        for tt in range(NT):
            if tt == 1:
                # A-projection weights: first needed by the skewed MM3
                # later this iteration; loading now keeps the tt=0 ramp
                # DMA queues clear for the startup-critical u/B/tables
                for kt in range(NK):
                    ta = wa.tile([P, DM], _f16, name=f"art{kt}",
                                 tag=f"art{kt}")
                    nc.sync.dma_start(ta[:], art_d[kt])
                    art[kt] = ta
                    ti = wa.tile([P, DM], _f16, name=f"ait{kt}",
                                 tag=f"ait{kt}")
                    nc.sync.dma_start(ti[:], ait_d[kt])
                    ait[kt] = ti
            ts = slice(tt * TT, (tt + 1) * TT)
            us = []
            for di in range(ND):
                t = ust.tile([P, TT], _f16, tag=f"us{di}")
                nc.sync.dma_start(t[:], ut_d[tt, di])
                us.append(t)
            hr2, hi2 = [], []
            for p in range(NP):
                kt0, kt1 = 2 * p, 2 * p + 1
                if tt == 0:
                    for kt in (kt0, kt1):
                        tb = wb.tile([P, DM], _f16, name=f"brw{kt}",
                                     tag=f"brw{kt}")
                        nc.sync.dma_start(tb[:], brt_d[kt])
                        brw[kt] = tb
                        ti = wb.tile([P, DM], _f16, name=f"biw{kt}",
                                     tag=f"biw{kt}")
                        nc.sync.dma_start(ti[:], bit_d[kt])
                        biw[kt] = ti
                c2 = tbl.tile([P, W], _f32, tag="c2")
                s2 = tbl.tile([P, W], _f32, tag="s2")
                nc.sync.dma_start(c2[:, :TT], cs_d[tt, kt0])
                nc.sync.dma_start(c2[:, TT:], cs_d[tt, kt1])
                nc.sync.dma_start(s2[:, :TT], sn_d[tt, kt0])
                nc.sync.dma_start(s2[:, TT:], sn_d[tt, kt1])

                sr2 = sbuf_s.tile([P, W], _f16, tag="sr2")
                si2 = sbuf_s.tile([P, W], _f16, tag="si2")
                for h, kt in ((0, kt0), (1, kt1)):
                    sl = slice(h * TT, (h + 1) * TT)
                    pbr = psA.tile([P, TT], _f32, tag="pbr")
                    pbi = psA.tile([P, TT], _f32, tag="pbi")
                    for di in range(ND):
                        dsl = slice(di * P, (di + 1) * P)
                        nc.tensor.matmul(pbr[:], brw[kt][:, dsl], us[di][:],
                                         start=(di == 0), stop=(di == ND - 1))
                        nc.tensor.matmul(pbi[:], biw[kt][:, dsl], us[di][:],
                                         start=(di == 0), stop=(di == ND - 1))
                    m1 = scr.tile([P, TT], _f32, tag="m1")
                    m2 = scr.tile([P, TT], _f32, tag="m2")
                    nc.vector.tensor_tensor(m1[:], c2[:, sl], pbr[:], _mul)
                    nc.vector.tensor_tensor(m2[:], s2[:, sl], pbi[:], _mul)
                    e = (h + 1) * TT
                    init_r = 0.0 if tt == 0 else prev_s[p][0][:, e - 1:e]
                    nc.vector.tensor_tensor_scan(sr2[:, sl], m1[:], m2[:],
                                                 init_r, _add, _add)
                    m3 = scr.tile([P, TT], _f32, tag="m1")
                    m4 = scr.tile([P, TT], _f32, tag="m2")
                    nc.vector.tensor_tensor(m3[:], c2[:, sl], pbi[:], _mul)
                    nc.vector.tensor_tensor(m4[:], s2[:, sl], pbr[:], _mul)
                    init_i = 0.0 if tt == 0 else prev_s[p][1][:, e - 1:e]
                    nc.vector.tensor_tensor_scan(si2[:, sl], m3[:], m4[:],
                                                 init_i, _add, _sub)

                # untwist on [P, 2*TT] in fp16 (DVE 2x mode); fp16
                # table copies generated on the otherwise-idle ACT engine
                c2h = scr.tile([P, W], _f16, tag="c2h", bufs=2)
                s2h = scr.tile([P, W], _f16, tag="s2h", bufs=2)
                nc.scalar.copy(c2h[:], c2[:])
                nc.scalar.copy(s2h[:], s2[:])
                w1 = scr.tile([P, W], _f16, tag="w1", bufs=1)
                w2 = scr.tile([P, W], _f16, tag="w2", bufs=1)
                hrp = hbuf.tile([P, W], _f16, name=f"hr2_{p}", tag=f"hr2_{p}")
                hip = hbuf.tile([P, W], _f16, name=f"hi2_{p}", tag=f"hi2_{p}")
                nc.vector.tensor_tensor(w1[:], c2h[:], sr2[:], _mul)
                nc.vector.tensor_tensor(w2[:], s2h[:], si2[:], _mul)
                nc.vector.tensor_tensor(hrp[:], w1[:], w2[:], _sub)
                w3 = scr.tile([P, W], _f16, tag="w3", bufs=1)
                w4 = scr.tile([P, W], _f16, tag="w4", bufs=1)
                nc.vector.tensor_tensor(w3[:], c2h[:], si2[:], _mul)
                nc.vector.tensor_tensor(w4[:], s2h[:], sr2[:], _mul)
                if tt == NT - 1:
                    nc.vector.tensor_tensor(hip[:], w3[:], w4[:], _add)
                else:
                    nc.gpsimd.tensor_tensor(hip[:], w3[:], w4[:], _add)
                hr2.append(hrp)
                hi2.append(hip)
                prev_s[p] = (sr2, si2)
                if tt > 0:
                    half = ND // NP
                    _emit_mm3(nc, psY, sbuf_s, art, ait, prev_hr2, prev_hi2,
                              yt_d, tt - 1, p * half, (p + 1) * half)
                if tt == NT - 1 and p == 1:
                    # pair-0 partial accumulations for the first half of the
                    # final MM3: PE runs these while DVE finishes pair 1
                    _tail_py = []
                    for di in range(ND // 2):
                        py = psY.tile([P, TT], _f32, tag="py")
                        for h, kt in ((0, 0), (1, 1)):
                            sl = slice(h * TT, (h + 1) * TT)
                            nc.tensor.matmul(
                                py[:], art[kt][:, di * P:(di + 1) * P],
                                hr2[0][:, sl], start=(h == 0), stop=False)
                            nc.tensor.matmul(
                                py[:], ait[kt][:, di * P:(di + 1) * P],
                                hi2[0][:, sl], start=False, stop=False)
                        _tail_py.append(py)


            prev_hr2, prev_hi2 = hr2, hi2
        # tail: di 0..3 already have pair-0 partials accumulated (emitted
        # inside the last pair loop below); finish them, then di 4..7
        tt = NT - 1
        for di in range(ND // 2):
            py = _tail_py[di]
            for h, kt in ((0, 2), (1, 3)):
                sl = slice(h * TT, (h + 1) * TT)
                nc.tensor.matmul(py[:], art[kt][:, di * P:(di + 1) * P],
                                 prev_hr2[1][:, sl], start=False, stop=False)
                nc.tensor.matmul(py[:], ait[kt][:, di * P:(di + 1) * P],
                                 prev_hi2[1][:, sl], start=False,
                                 stop=(h == 1))
            ysb = sbuf_s.tile([P, TT], _f32, tag="ysb", bufs=2)
            nc.scalar.copy(ysb[:], py[:])
            nc.sync.dma_start(yt_d[tt, di], ysb[:])
        _emit_mm3(nc, psY, sbuf_s, art, ait, prev_hr2, prev_hi2, yt_d,
                  NT - 1, ND // 2, ND)


def _build():
    nc = bacc.Bacc("TRN2", target_bir_lowering=False, debug=False,
                   num_devices=BATCH)
    ut_d = nc.dram_tensor("ut", [NT, ND, P, TT], _f16, kind="ExternalInput")
    brt_d = nc.dram_tensor("brt", [NK, P, DM], _f16, kind="ExternalInput")
    bit_d = nc.dram_tensor("bit", [NK, P, DM], _f16, kind="ExternalInput")
    art_d = nc.dram_tensor("art", [NK, P, DM], _f16, kind="ExternalInput")
    ait_d = nc.dram_tensor("ait", [NK, P, DM], _f16, kind="ExternalInput")
    cs_d = nc.dram_tensor("cs", [NT, NK, P, TT], _f32, kind="ExternalInput")
    sn_d = nc.dram_tensor("sn", [NT, NK, P, TT], _f32, kind="ExternalInput")
    yt_d = nc.dram_tensor("yt", [NT, ND, P, TT], _f32, kind="ExternalOutput")
    with tile.TileContext(nc) as tc:
        _emit(tc, nc, (ut_d, brt_d, bit_d, art_d, ait_d, cs_d, sn_d, yt_d))
    nc.compile()
    return nc


def _host_prep(a_params, B_w, C_w):
    """Fold the DFT into B/C and build phase tables (float64 on host)."""
    n, half = NSTATE, K
    a = a_params.astype(np.float64)
    a_full = np.zeros(n)
    a_full[1:half] = a[:half - 1]
    a_full[half + 1:] = -a[:half - 1][::-1]
    omega = np.fft.fft(a_full).imag
    theta = -2.0 * np.arctan(omega)          # lambda_k = e^{i theta_k}

    Bf = np.fft.fft(B_w.astype(np.float64), axis=0)[:half + 1]      # (513, d)
    G = np.conj(np.fft.fft(C_w.astype(np.float64), axis=1))[:, :half + 1]

    Br = np.empty((K, DM))
    Bi = np.empty((K, DM))
    Br[0], Bi[0] = Bf[0].real, Bf[half].real   # DC + Nyquist packed, theta=0
    Br[1:], Bi[1:] = Bf[1:half].real, Bf[1:half].imag
    Ar = np.empty((DM, K))
    Ai = np.empty((DM, K))
    Ar[:, 0] = (1.0 / n) * G[:, 0].real
    Ai[:, 0] = (1.0 / n) * G[:, half].real
    Ar[:, 1:] = (2.0 / n) * G[:, 1:half].real
    Ai[:, 1:] = -(2.0 / n) * G[:, 1:half].imag
    th = theta[:half].copy()
    th[0] = 0.0

    ang = np.outer(th, np.arange(SEQ, dtype=np.float64))   # (K, SEQ)
    f32 = np.float32

    def tiles_tt(m, p):       # (R, SEQ) -> (NT, R//p, p, TT) contiguous
        r = m.reshape(m.shape[0] // p, p, NT, TT)
        return np.ascontiguousarray(r.transpose(2, 0, 1, 3), dtype=f32)

    def tiles(m, p):          # (R, C) -> (R//p, p, C) contiguous f32
        return np.ascontiguousarray(m.reshape(m.shape[0] // p, p, m.shape[1]),
                                    dtype=f32)

    # packed B layout: [kt, p, di*128+j] = Br[kt*128+j, di*128+p]
    def bpack(M):                 # (K, DM) -> (NK, P, DM)
        r = M.reshape(NK, P, ND, P)          # [kt, j, di, p]
        return np.ascontiguousarray(r.transpose(0, 3, 2, 1).reshape(NK, P, DM),
                                    dtype=np.float32)

    return {
        "brt": bpack(Br).astype(np.float16),
        "bit": bpack(Bi).astype(np.float16),
        "art": tiles(Ar.T.copy(), P).astype(np.float16),
        "ait": tiles(Ai.T.copy(), P).astype(np.float16),
        "cs": tiles_tt(np.cos(ang), P),     # (NT, NK, 128, TT)
        "sn": tiles_tt(np.sin(ang), P),
    }


def _run(u, a_params, B_w, C_w, D, trace=False):
    global _COMPILED
    if _COMPILED is None:
        _COMPILED = _build()
    nc = _COMPILED
    shared = _host_prep(np.asarray(a_params), np.asarray(B_w), np.asarray(C_w))
    u = np.asarray(u)
    in_maps = []
    for b in range(BATCH):
        m = dict(shared)
        m["ut"] = np.ascontiguousarray(
            u[b].T.reshape(ND, P, NT, TT).transpose(2, 0, 1, 3),
            dtype=np.float16)
        in_maps.append(m)
    res = run_bass_kernel_spmd(nc, in_maps, core_ids=list(range(BATCH)),
                               trace=trace)
    y = np.empty((BATCH, SEQ, DM), dtype=np.float32)
    for b in range(BATCH):
        yt = res.results[b]["yt"]            # (NT, ND, P, TT)
        y[b] = yt.transpose(1, 2, 0, 3).reshape(DM, SEQ).T
    y += np.asarray(D)[None, None, :] * u
    return y, res


def kernel(u, a_params, B_w, C_w, D):
    y, _ = _run(u, a_params, B_w, C_w, D)
    return y
